# revision 2
# baseline (speedup 1.0000x reference)
"""HAN layer (3-metapath GAT + semantic attention) on 8 TRN2 NeuronCores.

Sharding: nodes partitioned 6250/core (graph parallel); edges sharded by
dst-node owner. Each core builds the full node-feature table
T = [er | el | feat] per metapath (h @ W' over all 50k nodes), then
processes its local dst nodes: one node per SBUF partition lane
(degree-sorted for load balance), edges gathered per "round" with
indirect DMA, attention+softmax per lane, aggregation via diagonal
matmuls accumulated in PSUM. Semantic attention uses a tiny AllReduce.
"""

import numpy as np
import ml_dtypes

import concourse.bass as bass
import concourse.tile as tile
from concourse import bacc, mybir
from concourse.bass_utils import run_bass_kernel_spmd
from concourse.masks import make_identity

N = 50000
E = 800000
P = 3
IN = 256
D = 64
SEM_H = 128
NEG = 0.2
NC_ = 8
NSH = N // NC_          # 6250 nodes per core
NT = (NSH + 127) // 128  # 49 node tiles per core
BF16 = mybir.dt.bfloat16
F32 = mybir.dt.float32
I32 = mybir.dt.int32


def _preprocess(h, srcs, dsts, W, attn_l, attn_r):
    # fused projection weights: per path cols [er_w | el_w | feat_w(64)]
    Wp = np.zeros((IN, P * 66), np.float32)
    for p in range(P):
        Wp[:, p * 66 + 0] = W[p] @ attn_r[p, 0]
        Wp[:, p * 66 + 1] = W[p] @ attn_l[p, 0]
        Wp[:, p * 66 + 2:p * 66 + 66] = W[p]
    hT = np.ascontiguousarray(h.T).astype(ml_dtypes.bfloat16)
    Wpb = Wp.astype(ml_dtypes.bfloat16)

    # per-core edge grids
    grids = [[None] * P for _ in range(NC_)]   # (lane, round) -> src
    masks = [[None] * P for _ in range(NC_)]
    perms = []
    degtot = np.zeros((NC_, NSH), np.int64)
    loc_all = [[None] * P for _ in range(NC_)]
    for p in range(P):
        own = dsts[p] // NSH
        for k in range(NC_):
            sel = own == k
            loc_all[k][p] = (srcs[p][sel].astype(np.int64),
                             (dsts[p][sel] - k * NSH).astype(np.int64))
            degtot[k] += np.bincount(loc_all[k][p][1], minlength=NSH)
    for k in range(NC_):
        perm = np.argsort(-degtot[k], kind="stable")  # high degree first
        perms.append(perm)
    Bv = np.zeros(NT, np.int64)
    for k in range(NC_):
        lane_of = np.empty(NSH, np.int64)
        lane_of[perms[k]] = np.arange(NSH)
        for p in range(P):
            s, d = loc_all[k][p]
            lane = lane_of[d]
            o = np.argsort(lane, kind="stable")
            lane_s, src_s = lane[o], s[o]
            starts = np.searchsorted(lane_s, np.arange(NSH))
            r = np.arange(len(lane_s)) - starts[lane_s]
            Bmax = int(r.max()) + 1 if len(r) else 1
            g = np.zeros((NT * 128, Bmax), np.int32)
            m = np.zeros((NT * 128, Bmax), ml_dtypes.bfloat16)
            g[lane_s, r] = src_s
            m[lane_s, r] = 1.0
            grids[k][p] = g
            masks[k][p] = m
            for v in range(NT):
                lo = v * 128
                cnt = r[(lane_s >= lo) & (lane_s < lo + 128)]
                bv = int(cnt.max()) + 1 if len(cnt) else 1
                Bv[v] = max(Bv[v], bv)
    Bv = [int(x) for x in Bv]
    Btot = int(sum(Bv))

    # assemble per-core device arrays
    ins = []
    for k in range(NC_):
        srcI = np.zeros((128, P * Btot), np.int32)
        mskI = np.zeros((128, P * Btot), ml_dtypes.bfloat16)
        for p in range(P):
            off = p * Btot
            c = 0
            for v in range(NT):
                b = Bv[v]
                gw = grids[k][p].shape[1]
                take = min(b, gw)
                srcI[:, off + c:off + c + take] = grids[k][p][v * 128:(v + 1) * 128, :take]
                mskI[:, off + c:off + c + take] = masks[k][p][v * 128:(v + 1) * 128, :take]
                c += b
        perm = perms[k]
        gids = np.full((128, NT), k * NSH, np.int32)
        rowI = np.full((128, NT), 2 * N, np.int32)
        nmsk = np.zeros((128, NT), np.float32)
        for v in range(NT):
            nn = min(128, NSH - v * 128)
            gids[:nn, v] = (k * NSH + perm[v * 128:v * 128 + nn]).astype(np.int32)
            rowI[:nn, v] = perm[v * 128:v * 128 + nn].astype(np.int32)
            nmsk[:nn, v] = 1.0
        ins.append(dict(srcI=srcI, mskI=mskI, gids=gids, rowI=rowI, nmsk=nmsk))
    return hT, Wpb, Bv, Btot, ins


def _build(Bv, Btot):
    Bmax = max(Bv)
    nc = bacc.Bacc("TRN2", target_bir_lowering=False, debug=False)
    hT = nc.dram_tensor("hT", [IN, N], BF16, kind="ExternalInput").ap()
    Wp = nc.dram_tensor("Wp", [IN, P * 66], BF16, kind="ExternalInput").ap()
    srcI = nc.dram_tensor("srcI", [128, P * Btot], I32, kind="ExternalInput").ap()
    mskI = nc.dram_tensor("mskI", [128, P * Btot], BF16, kind="ExternalInput").ap()
    gids = nc.dram_tensor("gids", [128, NT], I32, kind="ExternalInput").ap()
    rowI = nc.dram_tensor("rowI", [128, NT], I32, kind="ExternalInput").ap()
    nmsk = nc.dram_tensor("nmsk", [128, NT], F32, kind="ExternalInput").ap()
    W1 = nc.dram_tensor("W1", [D, SEM_H], F32, kind="ExternalInput").ap()
    b1 = nc.dram_tensor("b1", [SEM_H, 1], F32, kind="ExternalInput").ap()
    w2 = nc.dram_tensor("w2", [SEM_H, 1], F32, kind="ExternalInput").ap()
    out = nc.dram_tensor("out", [NSH, D], F32, kind="ExternalOutput").ap()
    T = nc.dram_tensor("T", [N, P * 66], BF16).ap()
    crin = nc.dram_tensor("crin", [1, 4], F32).ap()
    crout = nc.dram_tensor("crout", [1, 4], F32, addr_space="Shared").ap()

    with tile.TileContext(nc) as tc:
        with (
            tc.tile_pool(name="persist", bufs=1) as pp,
            tc.tile_pool(name="work", bufs=3) as wp,
            tc.tile_pool(name="gpool", bufs=3) as gp,
            tc.tile_pool(name="psA", bufs=2, space="PSUM") as psa,
            tc.tile_pool(name="psB", bufs=2, space="PSUM") as ps,
            tc.tile_pool(name="psS", bufs=1, space="PSUM") as ps1,
        ):
            # resident constants
            Wp0 = pp.tile([128, P * 66], BF16)
            Wp1 = pp.tile([128, P * 66], BF16)
            nc.sync.dma_start(Wp0[:], Wp[0:128, :])
            nc.sync.dma_start(Wp1[:], Wp[128:256, :])
            identF = pp.tile([128, 128], F32)
            make_identity(nc, identF[:])
            identB = pp.tile([128, 128], BF16)
            nc.vector.tensor_copy(identB[:], identF[:])
            W1sb = pp.tile([D, SEM_H], F32)
            nc.sync.dma_start(W1sb[:], W1[:])
            b1sb = pp.tile([SEM_H, 1], F32)
            nc.sync.dma_start(b1sb[:], b1[:])
            w2sb = pp.tile([SEM_H, 1], F32)
            nc.sync.dma_start(w2sb[:], w2[:])
            gid_t = pp.tile([128, NT], I32)
            nc.sync.dma_start(gid_t[:], gids[:])
            row_t = pp.tile([128, NT], I32)
            nc.sync.dma_start(row_t[:], rowI[:])
            nmsk_t = pp.tile([128, NT], F32)
            nc.sync.dma_start(nmsk_t[:], nmsk[:])
            zbuf = pp.tile([128, NT * P * D], F32)
            wbuf = pp.tile([128, P * NT], F32)
            onesc = pp.tile([128, 1], F32)
            nc.gpsimd.memset(onesc[:], 1.0)
            ones1 = pp.tile([1, 128], F32)
            nc.gpsimd.memset(ones1[:], 1.0)

            # ---- Phase A: T = hT.T @ Wp over all 50k nodes ----
            GRP = 8
            ntile_a = (N + 127) // 128
            t0 = 0
            while t0 < ntile_a:
                gtiles = min(GRP, ntile_a - t0)
                ncols = min(N - t0 * 128, gtiles * 128)
                h0 = wp.tile([128, GRP * 128], BF16, tag="h0")
                h1 = wp.tile([128, GRP * 128], BF16, tag="h1")
                nc.sync.dma_start(h0[:, :ncols], hT[0:128, t0 * 128:t0 * 128 + ncols])
                nc.sync.dma_start(h1[:, :ncols], hT[128:256, t0 * 128:t0 * 128 + ncols])
                for i in range(gtiles):
                    t = t0 + i
                    w = min(128, N - t * 128)
                    pa = psa.tile([128, P * 66], F32, tag="pa")
                    nc.tensor.matmul(out=pa[:w, :], lhsT=h0[:, i * 128:i * 128 + w],
                                     rhs=Wp0[:], start=True, stop=False)
                    nc.tensor.matmul(out=pa[:w, :], lhsT=h1[:, i * 128:i * 128 + w],
                                     rhs=Wp1[:], start=False, stop=True)
                    stg = wp.tile([128, P * 66], BF16, tag="stg")
                    nc.vector.tensor_copy(stg[:w, :], pa[:w, :])
                    nc.sync.dma_start(T[t * 128:t * 128 + w, :], stg[:w, :])
                t0 += gtiles

            # ---- Phase B: per metapath, per node tile ----
            offs = np.cumsum([0] + Bv)
            for p in range(P):
                for v in range(NT):
                    B = Bv[v]
                    c0 = p * Btot + int(offs[v])
                    si = wp.tile([128, Bmax], I32, tag="si")
                    nc.sync.dma_start(si[:, :B], srcI[:, c0:c0 + B])
                    mk = wp.tile([128, Bmax], BF16, tag="mk")
                    nc.sync.dma_start(mk[:, :B], mskI[:, c0:c0 + B])
                    G = gp.tile([128, Bmax, 66], BF16, tag="G")
                    for b in range(B):
                        nc.gpsimd.indirect_dma_start(
                            out=G[:, b, 0:65], out_offset=None, in_=T[:],
                            in_offset=bass.IndirectOffsetOnAxis(ap=si[:, b:b + 1], axis=0),
                            element_offset=p * 66 + 1)
                    # er for this tile's nodes (col0 of gathered row)
                    ert = wp.tile([128, 65], BF16, tag="ert")
                    nc.gpsimd.indirect_dma_start(
                        out=ert[:], out_offset=None, in_=T[:],
                        in_offset=bass.IndirectOffsetOnAxis(ap=gid_t[:, v:v + 1], axis=0),
                        element_offset=p * 66)
                    # ex = exp(leaky(el + er)) * mask   (f32 math)
                    Ef = wp.tile([128, Bmax], F32, tag="Ef")
                    nc.vector.tensor_tensor(out=Ef[:, :B], in0=G[:, 0:B, 0],
                                            in1=ert[:, 0:1].broadcast_to([128, B]),
                                            op=mybir.AluOpType.add)
                    Lk = wp.tile([128, Bmax], F32, tag="Lk")
                    nc.vector.tensor_scalar_mul(Lk[:, :B], Ef[:, :B], NEG)
                    nc.vector.tensor_tensor(out=Ef[:, :B], in0=Ef[:, :B], in1=Lk[:, :B],
                                            op=mybir.AluOpType.max)
                    EXf = wp.tile([128, Bmax], F32, tag="EXf")
                    nc.scalar.activation(EXf[:, :B], Ef[:, :B],
                                         mybir.ActivationFunctionType.Exp)
                    EXb = wp.tile([128, Bmax], BF16, tag="EXb")
                    nc.vector.tensor_tensor(out=EXb[:, :B], in0=EXf[:, :B], in1=mk[:, :B],
                                            op=mybir.AluOpType.mult)
                    den = wp.tile([128, 1], F32, tag="den")
                    nc.vector.reduce_sum(den[:], EXb[:, 0:B], axis=mybir.AxisListType.X)
                    # MT[:, b, :] = diag-scale matrices
                    MT = gp.tile([128, Bmax, 128], BF16, tag="MT")
                    nc.vector.tensor_tensor(
                        out=MT[:, 0:B, :],
                        in0=identB[:, None, :].broadcast_to([128, B, 128]),
                        in1=EXb[:, 0:B, None].broadcast_to([128, B, 128]),
                        op=mybir.AluOpType.mult)
                    agg = ps.tile([128, D], F32, tag="agg")
                    for b in range(B):
                        nc.tensor.matmul(out=agg[:], lhsT=MT[:, b, :], rhs=G[:, b, 1:65],
                                         start=(b == 0), stop=(b == B - 1))
                    nc.vector.tensor_scalar_max(den[:], den[:], 1e-9)
                    rec = wp.tile([128, 1], F32, tag="rec")
                    nc.vector.reciprocal(rec[:], den[:])
                    zt = wp.tile([128, D], F32, tag="zt")
                    nc.scalar.activation(zt[:], agg[:], mybir.ActivationFunctionType.Copy,
                                         scale=rec[:])
                    # elu: max(x,0) + exp(min(x,0)) - 1
                    t1 = wp.tile([128, D], F32, tag="t1")
                    nc.vector.tensor_scalar_min(t1[:], zt[:], 0.0)
                    t2 = wp.tile([128, D], F32, tag="t2")
                    nc.scalar.activation(t2[:], t1[:], mybir.ActivationFunctionType.Exp)
                    t3 = wp.tile([128, D], F32, tag="t3")
                    nc.vector.tensor_scalar_max(t3[:], zt[:], 0.0)
                    nc.vector.tensor_tensor(out=t2[:], in0=t2[:], in1=t3[:],
                                            op=mybir.AluOpType.add)
                    zslot = zbuf[:, (v * P + p) * D:(v * P + p + 1) * D]
                    nc.vector.tensor_scalar_add(zslot, t2[:], -1.0)
                    # semantic score w = tanh(z @ W1 + b1) @ w2
                    pt = ps1.tile([D, 128], F32, tag="ps_sem")
                    nc.tensor.transpose(out=pt[:], in_=zslot, identity=identF[:])
                    ztT = wp.tile([D, 128], F32, tag="ztT")
                    nc.vector.tensor_copy(ztT[:], pt[:])
                    ph = ps1.tile([SEM_H, 128], F32, tag="ps_sem")
                    nc.tensor.matmul(out=ph[:], lhsT=W1sb[:], rhs=ztT[:],
                                     start=True, stop=True)
                    th = wp.tile([SEM_H, 128], F32, tag="th")
                    nc.scalar.activation(th[:], ph[:], mybir.ActivationFunctionType.Tanh,
                                         bias=b1sb[:])
                    pw = ps1.tile([128, 1], F32, tag="ps_small")
                    nc.tensor.matmul(out=pw[:], lhsT=th[:], rhs=w2sb[:],
                                     start=True, stop=True)
                    nc.vector.tensor_copy(wbuf[:, p * NT + v:p * NT + v + 1], pw[:])

            # ---- semantic softmax over paths (global mean via AllReduce) ----
            wm = pp.tile([128, P * NT], F32)
            nc.vector.tensor_tensor(
                out=wm[:].rearrange("q (p v) -> q p v", p=P),
                in0=wbuf[:].rearrange("q (p v) -> q p v", p=P),
                in1=nmsk_t[:, None, :].broadcast_to([128, P, NT]),
                op=mybir.AluOpType.mult)
            ws3 = pp.tile([128, P], F32)
            nc.vector.reduce_sum(ws3[:, :, None], wm[:].rearrange("q (p v) -> q p v", p=P),
                                 axis=mybir.AxisListType.X)
            pt3 = ps1.tile([1, P], F32, tag="ps_small")
            nc.tensor.matmul(out=pt3[:], lhsT=onesc[:], rhs=ws3[:], start=True, stop=True)
            sb4 = pp.tile([1, 4], F32)
            nc.gpsimd.memset(sb4[:], 0.0)
            nc.vector.tensor_copy(sb4[:, 0:P], pt3[:])
            nc.sync.dma_start(crin[:], sb4[:])
            nc.gpsimd.collective_compute(
                "AllReduce", mybir.AluOpType.add,
                replica_groups=[list(range(NC_))],
                ins=[crin[:]], outs=[crout[:]])
            ar4 = pp.tile([1, 4], F32)
            nc.sync.dma_start(ar4[:], crout[:])
            ex3 = pp.tile([1, P], F32)
            nc.scalar.activation(ex3[:], ar4[:, 0:P],
                                 mybir.ActivationFunctionType.Exp, scale=1.0 / N)
            ssum = pp.tile([1, 1], F32)
            nc.vector.reduce_sum(ssum[:], ex3[:], axis=mybir.AxisListType.X)
            rs = pp.tile([1, 1], F32)
            nc.vector.reciprocal(rs[:], ssum[:])
            beta = pp.tile([1, P], F32)
            nc.vector.tensor_tensor(out=beta[:], in0=ex3[:],
                                    in1=rs[:].broadcast_to([1, P]),
                                    op=mybir.AluOpType.mult)
            pb = ps1.tile([128, P], F32, tag="ps_small")
            nc.tensor.matmul(out=pb[:], lhsT=ones1[:], rhs=beta[:], start=True, stop=True)
            betab = pp.tile([128, P], F32)
            nc.vector.tensor_copy(betab[:], pb[:])

            # ---- final combine + scatter to output rows ----
            for v in range(NT):
                o = wp.tile([128, D], F32, tag="o")
                z0 = zbuf[:, (v * P + 0) * D:(v * P + 1) * D]
                z1 = zbuf[:, (v * P + 1) * D:(v * P + 2) * D]
                z2 = zbuf[:, (v * P + 2) * D:(v * P + 3) * D]
                nc.vector.tensor_tensor(out=o[:], in0=z0,
                                        in1=betab[:, 0:1].broadcast_to([128, D]),
                                        op=mybir.AluOpType.mult)
                tt = wp.tile([128, D], F32, tag="tt")
                nc.vector.tensor_tensor(out=tt[:], in0=z1,
                                        in1=betab[:, 1:2].broadcast_to([128, D]),
                                        op=mybir.AluOpType.mult)
                nc.vector.tensor_tensor(out=o[:], in0=o[:], in1=tt[:],
                                        op=mybir.AluOpType.add)
                nc.vector.tensor_tensor(out=tt[:], in0=z2,
                                        in1=betab[:, 2:3].broadcast_to([128, D]),
                                        op=mybir.AluOpType.mult)
                nc.vector.tensor_tensor(out=o[:], in0=o[:], in1=tt[:],
                                        op=mybir.AluOpType.add)
                nc.gpsimd.indirect_dma_start(
                    out=out[:], out_offset=bass.IndirectOffsetOnAxis(
                        ap=row_t[:, v:v + 1], axis=0),
                    in_=o[:], in_offset=None,
                    bounds_check=NSH - 1, oob_is_err=False)
    nc.compile()
    return nc


def kernel(h, src0, dst0, src1, dst1, src2, dst2, W, attn_l, attn_r,
           sem_W1, sem_b1, sem_w2):
    import time as _t
    _ts = _t.perf_counter()
    h = np.asarray(h, np.float32)
    W = np.asarray(W, np.float32)
    attn_l = np.asarray(attn_l, np.float32)
    attn_r = np.asarray(attn_r, np.float32)
    srcs = [np.asarray(s, np.int32) for s in (src0, src1, src2)]
    dsts = [np.asarray(d, np.int32) for d in (dst0, dst1, dst2)]
    hT, Wpb, Bv, Btot, pins = _preprocess(h, srcs, dsts, W, attn_l, attn_r)
    print(f"[kern] preprocess: {_t.perf_counter()-_ts:.2f}s", flush=True)
    _ts2 = _t.perf_counter()
    nc = _build(Bv, Btot)
    print(f"[kern] build+compile(client): {_t.perf_counter()-_ts2:.2f}s  Btot={Btot}", flush=True)
    W1v = np.asarray(sem_W1, np.float32)
    b1v = np.asarray(sem_b1, np.float32).reshape(SEM_H, 1)
    w2v = np.asarray(sem_w2, np.float32).reshape(SEM_H, 1)
    in_maps = []
    for k in range(NC_):
        d = pins[k]
        in_maps.append({
            "hT": hT, "Wp": Wpb, "srcI": d["srcI"], "mskI": d["mskI"],
            "gids": d["gids"], "rowI": d["rowI"], "nmsk": d["nmsk"],
            "W1": W1v, "b1": b1v, "w2": w2v,
        })
    import time as _t
    _t0 = _t.perf_counter()
    res = run_bass_kernel_spmd(nc, in_maps, core_ids=list(range(NC_)))
    global LAST_WALL_NS
    LAST_WALL_NS = (_t.perf_counter() - _t0) * 1e9
    return np.concatenate([res.results[k]["out"] for k in range(NC_)], axis=0)


LAST_WALL_NS = 0.0



# revision 3
# speedup vs baseline: 9.5548x; 9.5548x over previous
"""HAN layer (3-metapath GAT + semantic attention) on 8 TRN2 NeuronCores.

Sharding: nodes partitioned 6250/core (graph parallel); edges sharded by
dst-node owner. Each core builds the full node-feature table
T = [er | el | feat] per metapath (h @ W' over all 50k nodes), then
processes its local dst nodes: one node per SBUF partition lane
(degree-sorted for load balance), edges gathered per "round" with
indirect DMA, attention+softmax per lane, aggregation via diagonal
matmuls accumulated in PSUM. Semantic attention uses a tiny AllReduce.
"""

import numpy as np
import ml_dtypes

import concourse.bass as bass
import concourse.tile as tile
from concourse import bacc, mybir
from concourse.bass_utils import run_bass_kernel_spmd
from concourse.masks import make_identity

N = 50000
E = 800000
P = 3
IN = 256
D = 64
SEM_H = 128
NEG = 0.2
NC_ = 8
NSH = N // NC_          # 6250 nodes per core
NT = (NSH + 127) // 128  # 49 node tiles per core
BF16 = mybir.dt.bfloat16
F32 = mybir.dt.float32
I32 = mybir.dt.int32


def _preprocess(h, srcs, dsts, W, attn_l, attn_r):
    # fused projection weights: per path cols [er_w | el_w | feat_w(64)]
    Wp = np.zeros((IN, P * 66), np.float32)
    for p in range(P):
        Wp[:, p * 66 + 0] = W[p] @ attn_r[p, 0]
        Wp[:, p * 66 + 1] = W[p] @ attn_l[p, 0]
        Wp[:, p * 66 + 2:p * 66 + 66] = W[p]
    hT = np.ascontiguousarray(h.T).astype(ml_dtypes.bfloat16)
    Wpb = Wp.astype(ml_dtypes.bfloat16)

    # per-core edge grids
    grids = [[None] * P for _ in range(NC_)]   # (lane, round) -> src
    masks = [[None] * P for _ in range(NC_)]
    perms = []
    degtot = np.zeros((NC_, NSH), np.int64)
    loc_all = [[None] * P for _ in range(NC_)]
    for p in range(P):
        own = dsts[p] // NSH
        for k in range(NC_):
            sel = own == k
            loc_all[k][p] = (srcs[p][sel].astype(np.int64),
                             (dsts[p][sel] - k * NSH).astype(np.int64))
            degtot[k] += np.bincount(loc_all[k][p][1], minlength=NSH)
    for k in range(NC_):
        perm = np.argsort(-degtot[k], kind="stable")  # high degree first
        perms.append(perm)
    Bv = np.zeros(NT, np.int64)
    for k in range(NC_):
        lane_of = np.empty(NSH, np.int64)
        lane_of[perms[k]] = np.arange(NSH)
        for p in range(P):
            s, d = loc_all[k][p]
            lane = lane_of[d]
            o = np.argsort(lane, kind="stable")
            lane_s, src_s = lane[o], s[o]
            starts = np.searchsorted(lane_s, np.arange(NSH))
            r = np.arange(len(lane_s)) - starts[lane_s]
            Bmax = int(r.max()) + 1 if len(r) else 1
            g = np.zeros((NT * 128, Bmax), np.int32)
            m = np.zeros((NT * 128, Bmax), ml_dtypes.bfloat16)
            g[lane_s, r] = src_s
            m[lane_s, r] = 1.0
            grids[k][p] = g
            masks[k][p] = m
            for v in range(NT):
                lo = v * 128
                cnt = r[(lane_s >= lo) & (lane_s < lo + 128)]
                bv = int(cnt.max()) + 1 if len(cnt) else 1
                Bv[v] = max(Bv[v], bv)
    Bv = [int(x) for x in Bv]
    Btot = int(sum(Bv))

    # assemble per-core device arrays
    ins = []
    for k in range(NC_):
        srcI = np.zeros((128, P * Btot), np.int32)
        mskI = np.zeros((128, P * Btot), ml_dtypes.bfloat16)
        for p in range(P):
            off = p * Btot
            c = 0
            for v in range(NT):
                b = Bv[v]
                gw = grids[k][p].shape[1]
                take = min(b, gw)
                srcI[:, off + c:off + c + take] = grids[k][p][v * 128:(v + 1) * 128, :take]
                mskI[:, off + c:off + c + take] = masks[k][p][v * 128:(v + 1) * 128, :take]
                c += b
        perm = perms[k]
        gids = np.full((128, NT), k * NSH, np.int32)
        rowI = np.full((128, NT), 2 * N, np.int32)
        nmsk = np.zeros((128, NT), np.float32)
        for v in range(NT):
            nn = min(128, NSH - v * 128)
            gids[:nn, v] = (k * NSH + perm[v * 128:v * 128 + nn]).astype(np.int32)
            rowI[:nn, v] = perm[v * 128:v * 128 + nn].astype(np.int32)
            nmsk[:nn, v] = 1.0
        ins.append(dict(srcI=srcI, mskI=mskI, gids=gids, rowI=rowI, nmsk=nmsk))
    return hT, Wpb, Bv, Btot, ins


def _build(Bv, Btot):
    Bmax = max(Bv)
    nc = bacc.Bacc("TRN2", target_bir_lowering=False, debug=False)
    hT = nc.dram_tensor("hT", [IN, N], BF16, kind="ExternalInput").ap()
    Wp = nc.dram_tensor("Wp", [IN, P * 66], BF16, kind="ExternalInput").ap()
    srcI = nc.dram_tensor("srcI", [128, P * Btot], I32, kind="ExternalInput").ap()
    mskI = nc.dram_tensor("mskI", [128, P * Btot], BF16, kind="ExternalInput").ap()
    gids = nc.dram_tensor("gids", [128, NT], I32, kind="ExternalInput").ap()
    rowI = nc.dram_tensor("rowI", [128, NT], I32, kind="ExternalInput").ap()
    nmsk = nc.dram_tensor("nmsk", [128, NT], F32, kind="ExternalInput").ap()
    W1 = nc.dram_tensor("W1", [D, SEM_H], F32, kind="ExternalInput").ap()
    b1 = nc.dram_tensor("b1", [SEM_H, 1], F32, kind="ExternalInput").ap()
    w2 = nc.dram_tensor("w2", [SEM_H, 1], F32, kind="ExternalInput").ap()
    out = nc.dram_tensor("out", [NSH, D], F32, kind="ExternalOutput").ap()
    T = nc.dram_tensor("T", [N, P * 66], BF16).ap()
    crin = nc.dram_tensor("crin", [1, 4], F32).ap()
    crout = nc.dram_tensor("crout", [1, 4], F32, addr_space="Shared").ap()

    with tile.TileContext(nc) as tc:
        with (
            tc.tile_pool(name="persist", bufs=1) as pp,
            tc.tile_pool(name="work", bufs=3) as wp,
            tc.tile_pool(name="gpool", bufs=3) as gp,
            tc.tile_pool(name="psA", bufs=2, space="PSUM") as psa,
            tc.tile_pool(name="psB", bufs=2, space="PSUM") as ps,
            tc.tile_pool(name="psS", bufs=1, space="PSUM") as ps1,
        ):
            # resident constants
            Wp0 = pp.tile([128, P * 66], BF16)
            Wp1 = pp.tile([128, P * 66], BF16)
            nc.sync.dma_start(Wp0[:], Wp[0:128, :])
            nc.sync.dma_start(Wp1[:], Wp[128:256, :])
            identF = pp.tile([128, 128], F32)
            make_identity(nc, identF[:])
            identB = pp.tile([128, 128], BF16)
            nc.vector.tensor_copy(identB[:], identF[:])
            W1sb = pp.tile([D, SEM_H], F32)
            nc.sync.dma_start(W1sb[:], W1[:])
            b1sb = pp.tile([SEM_H, 1], F32)
            nc.sync.dma_start(b1sb[:], b1[:])
            w2sb = pp.tile([SEM_H, 1], F32)
            nc.sync.dma_start(w2sb[:], w2[:])
            gid_t = pp.tile([128, NT], I32)
            nc.sync.dma_start(gid_t[:], gids[:])
            row_t = pp.tile([128, NT], I32)
            nc.sync.dma_start(row_t[:], rowI[:])
            nmsk_t = pp.tile([128, NT], F32)
            nc.sync.dma_start(nmsk_t[:], nmsk[:])
            zbuf = pp.tile([128, NT * P * D], F32)
            wbuf = pp.tile([128, P * NT], F32)
            onesc = pp.tile([128, 1], F32)
            nc.gpsimd.memset(onesc[:], 1.0)
            ones1 = pp.tile([1, 128], F32)
            nc.gpsimd.memset(ones1[:], 1.0)

            # ---- Phase A: T = hT.T @ Wp over all 50k nodes ----
            GRP = 8
            ntile_a = (N + 127) // 128
            t0 = 0
            while t0 < ntile_a:
                gtiles = min(GRP, ntile_a - t0)
                ncols = min(N - t0 * 128, gtiles * 128)
                h0 = wp.tile([128, GRP * 128], BF16, tag="h0")
                h1 = wp.tile([128, GRP * 128], BF16, tag="h1")
                nc.sync.dma_start(h0[:, :ncols], hT[0:128, t0 * 128:t0 * 128 + ncols])
                nc.sync.dma_start(h1[:, :ncols], hT[128:256, t0 * 128:t0 * 128 + ncols])
                for i in range(gtiles):
                    t = t0 + i
                    w = min(128, N - t * 128)
                    pa = psa.tile([128, P * 66], F32, tag="pa")
                    nc.tensor.matmul(out=pa[:w, :], lhsT=h0[:, i * 128:i * 128 + w],
                                     rhs=Wp0[:], start=True, stop=False)
                    nc.tensor.matmul(out=pa[:w, :], lhsT=h1[:, i * 128:i * 128 + w],
                                     rhs=Wp1[:], start=False, stop=True)
                    stg = wp.tile([128, P * 66], BF16, tag="stg")
                    nc.vector.tensor_copy(stg[:w, :], pa[:w, :])
                    nc.sync.dma_start(T[t * 128:t * 128 + w, :], stg[:w, :])
                t0 += gtiles

            # ---- Phase B: per metapath, per node tile ----
            offs = np.cumsum([0] + Bv)
            for p in range(P):
                for v in range(NT):
                    B = Bv[v]
                    c0 = p * Btot + int(offs[v])
                    si = wp.tile([128, Bmax], I32, tag="si")
                    nc.sync.dma_start(si[:, :B], srcI[:, c0:c0 + B])
                    mk = wp.tile([128, Bmax], BF16, tag="mk")
                    nc.sync.dma_start(mk[:, :B], mskI[:, c0:c0 + B])
                    G = gp.tile([128, Bmax, 66], BF16, tag="G")
                    for b in range(B):
                        nc.gpsimd.indirect_dma_start(
                            out=G[:, b, 0:65], out_offset=None, in_=T[:],
                            in_offset=bass.IndirectOffsetOnAxis(ap=si[:, b:b + 1], axis=0),
                            element_offset=p * 66 + 1)
                    # er for this tile's nodes (col0 of gathered row)
                    ert = wp.tile([128, 65], BF16, tag="ert")
                    nc.gpsimd.indirect_dma_start(
                        out=ert[:], out_offset=None, in_=T[:],
                        in_offset=bass.IndirectOffsetOnAxis(ap=gid_t[:, v:v + 1], axis=0),
                        element_offset=p * 66)
                    # ex = exp(leaky(el + er)) * mask   (f32 math)
                    Ef = wp.tile([128, Bmax], F32, tag="Ef")
                    nc.vector.tensor_tensor(out=Ef[:, :B], in0=G[:, 0:B, 0],
                                            in1=ert[:, 0:1].broadcast_to([128, B]),
                                            op=mybir.AluOpType.add)
                    Lk = wp.tile([128, Bmax], F32, tag="Lk")
                    nc.vector.tensor_scalar_mul(Lk[:, :B], Ef[:, :B], NEG)
                    nc.vector.tensor_tensor(out=Ef[:, :B], in0=Ef[:, :B], in1=Lk[:, :B],
                                            op=mybir.AluOpType.max)
                    EXf = wp.tile([128, Bmax], F32, tag="EXf")
                    nc.scalar.activation(EXf[:, :B], Ef[:, :B],
                                         mybir.ActivationFunctionType.Exp)
                    EXb = wp.tile([128, Bmax], BF16, tag="EXb")
                    nc.vector.tensor_tensor(out=EXb[:, :B], in0=EXf[:, :B], in1=mk[:, :B],
                                            op=mybir.AluOpType.mult)
                    den = wp.tile([128, 1], F32, tag="den")
                    nc.vector.reduce_sum(den[:], EXb[:, 0:B], axis=mybir.AxisListType.X)
                    # MT[:, b, :] = diag-scale matrices
                    MT = gp.tile([128, Bmax, 128], BF16, tag="MT")
                    nc.vector.tensor_tensor(
                        out=MT[:, 0:B, :],
                        in0=identB[:, None, :].broadcast_to([128, B, 128]),
                        in1=EXb[:, 0:B, None].broadcast_to([128, B, 128]),
                        op=mybir.AluOpType.mult)
                    agg = ps.tile([128, D], F32, tag="agg")
                    for b in range(B):
                        nc.tensor.matmul(out=agg[:], lhsT=MT[:, b, :], rhs=G[:, b, 1:65],
                                         start=(b == 0), stop=(b == B - 1))
                    nc.vector.tensor_scalar_max(den[:], den[:], 1e-9)
                    rec = wp.tile([128, 1], F32, tag="rec")
                    nc.vector.reciprocal(rec[:], den[:])
                    zt = wp.tile([128, D], F32, tag="zt")
                    nc.scalar.activation(zt[:], agg[:], mybir.ActivationFunctionType.Copy,
                                         scale=rec[:])
                    # elu: max(x,0) + exp(min(x,0)) - 1
                    t1 = wp.tile([128, D], F32, tag="t1")
                    nc.vector.tensor_scalar_min(t1[:], zt[:], 0.0)
                    t2 = wp.tile([128, D], F32, tag="t2")
                    nc.scalar.activation(t2[:], t1[:], mybir.ActivationFunctionType.Exp)
                    t3 = wp.tile([128, D], F32, tag="t3")
                    nc.vector.tensor_scalar_max(t3[:], zt[:], 0.0)
                    nc.vector.tensor_tensor(out=t2[:], in0=t2[:], in1=t3[:],
                                            op=mybir.AluOpType.add)
                    zslot = zbuf[:, (v * P + p) * D:(v * P + p + 1) * D]
                    nc.vector.tensor_scalar_add(zslot, t2[:], -1.0)
                    # semantic score w = tanh(z @ W1 + b1) @ w2
                    pt = ps1.tile([D, 128], F32, tag="ps_sem")
                    nc.tensor.transpose(out=pt[:], in_=zslot, identity=identF[:])
                    ztT = wp.tile([D, 128], F32, tag="ztT")
                    nc.vector.tensor_copy(ztT[:], pt[:])
                    ph = ps1.tile([SEM_H, 128], F32, tag="ps_sem")
                    nc.tensor.matmul(out=ph[:], lhsT=W1sb[:], rhs=ztT[:],
                                     start=True, stop=True)
                    th = wp.tile([SEM_H, 128], F32, tag="th")
                    nc.scalar.activation(th[:], ph[:], mybir.ActivationFunctionType.Tanh,
                                         bias=b1sb[:])
                    pw = ps1.tile([128, 1], F32, tag="ps_small")
                    nc.tensor.matmul(out=pw[:], lhsT=th[:], rhs=w2sb[:],
                                     start=True, stop=True)
                    nc.vector.tensor_copy(wbuf[:, p * NT + v:p * NT + v + 1], pw[:])

            # ---- semantic softmax over paths (global mean via AllReduce) ----
            wm = pp.tile([128, P * NT], F32)
            nc.vector.tensor_tensor(
                out=wm[:].rearrange("q (p v) -> q p v", p=P),
                in0=wbuf[:].rearrange("q (p v) -> q p v", p=P),
                in1=nmsk_t[:, None, :].broadcast_to([128, P, NT]),
                op=mybir.AluOpType.mult)
            ws3 = pp.tile([128, P], F32)
            nc.vector.reduce_sum(ws3[:, :, None], wm[:].rearrange("q (p v) -> q p v", p=P),
                                 axis=mybir.AxisListType.X)
            pt3 = ps1.tile([1, P], F32, tag="ps_small")
            nc.tensor.matmul(out=pt3[:], lhsT=onesc[:], rhs=ws3[:], start=True, stop=True)
            sb4 = pp.tile([1, 4], F32)
            nc.gpsimd.memset(sb4[:], 0.0)
            nc.vector.tensor_copy(sb4[:, 0:P], pt3[:])
            nc.sync.dma_start(crin[:], sb4[:])
            nc.gpsimd.collective_compute(
                "AllReduce", mybir.AluOpType.add,
                replica_groups=[list(range(NC_))],
                ins=[crin[:]], outs=[crout[:]])
            ar4 = pp.tile([1, 4], F32)
            nc.sync.dma_start(ar4[:], crout[:])
            ex3 = pp.tile([1, P], F32)
            nc.scalar.activation(ex3[:], ar4[:, 0:P],
                                 mybir.ActivationFunctionType.Exp, scale=1.0 / N)
            ssum = pp.tile([1, 1], F32)
            nc.vector.reduce_sum(ssum[:], ex3[:], axis=mybir.AxisListType.X)
            rs = pp.tile([1, 1], F32)
            nc.vector.reciprocal(rs[:], ssum[:])
            beta = pp.tile([1, P], F32)
            nc.vector.tensor_tensor(out=beta[:], in0=ex3[:],
                                    in1=rs[:].broadcast_to([1, P]),
                                    op=mybir.AluOpType.mult)
            pb = ps1.tile([128, P], F32, tag="ps_small")
            nc.tensor.matmul(out=pb[:], lhsT=ones1[:], rhs=beta[:], start=True, stop=True)
            betab = pp.tile([128, P], F32)
            nc.vector.tensor_copy(betab[:], pb[:])

            # ---- final combine + scatter to output rows ----
            for v in range(NT):
                o = wp.tile([128, D], F32, tag="o")
                z0 = zbuf[:, (v * P + 0) * D:(v * P + 1) * D]
                z1 = zbuf[:, (v * P + 1) * D:(v * P + 2) * D]
                z2 = zbuf[:, (v * P + 2) * D:(v * P + 3) * D]
                nc.vector.tensor_tensor(out=o[:], in0=z0,
                                        in1=betab[:, 0:1].broadcast_to([128, D]),
                                        op=mybir.AluOpType.mult)
                tt = wp.tile([128, D], F32, tag="tt")
                nc.vector.tensor_tensor(out=tt[:], in0=z1,
                                        in1=betab[:, 1:2].broadcast_to([128, D]),
                                        op=mybir.AluOpType.mult)
                nc.vector.tensor_tensor(out=o[:], in0=o[:], in1=tt[:],
                                        op=mybir.AluOpType.add)
                nc.vector.tensor_tensor(out=tt[:], in0=z2,
                                        in1=betab[:, 2:3].broadcast_to([128, D]),
                                        op=mybir.AluOpType.mult)
                nc.vector.tensor_tensor(out=o[:], in0=o[:], in1=tt[:],
                                        op=mybir.AluOpType.add)
                nc.gpsimd.indirect_dma_start(
                    out=out[:], out_offset=bass.IndirectOffsetOnAxis(
                        ap=row_t[:, v:v + 1], axis=0),
                    in_=o[:], in_offset=None,
                    bounds_check=NSH - 1, oob_is_err=False)
    nc.compile()
    return nc


def kernel(h, src0, dst0, src1, dst1, src2, dst2, W, attn_l, attn_r,
           sem_W1, sem_b1, sem_w2):
    import time as _t
    _ts = _t.perf_counter()
    h = np.asarray(h, np.float32)
    W = np.asarray(W, np.float32)
    attn_l = np.asarray(attn_l, np.float32)
    attn_r = np.asarray(attn_r, np.float32)
    srcs = [np.asarray(s, np.int32) for s in (src0, src1, src2)]
    dsts = [np.asarray(d, np.int32) for d in (dst0, dst1, dst2)]
    hT, Wpb, Bv, Btot, pins = _preprocess(h, srcs, dsts, W, attn_l, attn_r)
    print(f"[kern] preprocess: {_t.perf_counter()-_ts:.2f}s", flush=True)
    _ts2 = _t.perf_counter()
    nc = _build(Bv, Btot)
    print(f"[kern] build+compile(client): {_t.perf_counter()-_ts2:.2f}s  Btot={Btot}", flush=True)
    W1v = np.asarray(sem_W1, np.float32)
    b1v = np.asarray(sem_b1, np.float32).reshape(SEM_H, 1)
    w2v = np.asarray(sem_w2, np.float32).reshape(SEM_H, 1)
    in_maps = []
    for k in range(NC_):
        d = pins[k]
        in_maps.append({
            "hT": hT, "Wp": Wpb, "srcI": d["srcI"], "mskI": d["mskI"],
            "gids": d["gids"], "rowI": d["rowI"], "nmsk": d["nmsk"],
            "W1": W1v, "b1": b1v, "w2": w2v,
        })
    import time as _t
    _t0 = _t.perf_counter()
    res = run_bass_kernel_spmd(nc, in_maps, core_ids=list(range(NC_)))
    global LAST_WALL_NS
    LAST_WALL_NS = (_t.perf_counter() - _t0) * 1e9
    try:
        print(f"[kern] device exec_time_ns: {res.exec_time_ns}", flush=True)
    except Exception:
        pass
    return np.concatenate([res.results[k]["out"] for k in range(NC_)], axis=0)


LAST_WALL_NS = 0.0

